# revision 25
# baseline (speedup 1.0000x reference)
"""Dense all-expert MoE (SwiGLU) kernel for Trainium2, expert-parallel over 8 cores.

Computes: out = sum_e silu(x @ Wg[e]) * (x @ Wu[e]) @ Wd[e]
with x: [B=2, S=2048, H=1024], Wg/Wu: [8, 1024, 4096], Wd: [8, 4096, 1024].

Sharding: expert-parallel. Core e gets expert e's weights plus the full token
set; each core produces a partial [T, H] output which the host sums.

Per-core kernel (bf16 matmul inputs, fp32 PSUM accumulation):
  stage A: hT[f, :, tokens] = silu(Wg_f^T @ xT) * (Wu_f^T @ xT)   (F on partitions)
  stage B: out[tokens, h]  += hT[f]^T @ Wd_f                      (tokens on partitions)
Host pre-lays-out all operands so every DMA is wide and contiguous:
  xT  [KB=8, 128, T]      xT[k, p, t]    = x[t, 128k+p]          (bf16)
  wg  [FB=32, 128, 1024]  wg[f, p, k*128+m] = Wg[128k+p, 128f+m] (bf16)
  wu  same layout as wg
  wd  [FB=32, 128, 1024]  wd[f, p, h]    = Wd[128f+p, h]         (bf16)
"""

import numpy as np
import ml_dtypes

T = 4096          # B*S tokens
H = 1024          # hidden
F = 4096          # ffn
E = 8             # experts
N_CORES = 8
TB = 1024         # tokens per block
NT = T // TB      # 4 token blocks
KB = H // 128     # 8 hidden slices
FB = F // 128     # 32 ffn slices

_CACHE = {}


def _build_module():
    from contextlib import ExitStack

    import concourse.bass as bass
    import concourse.mybir as mybir
    import concourse.tile as tile
    from concourse import bacc

    f32 = mybir.dt.float32
    bf16 = mybir.dt.bfloat16

    nc = bacc.Bacc(
        "TRN2",
        target_bir_lowering=False,
        debug=False,
        enable_asserts=False,
        num_devices=N_CORES,
    )

    xT = nc.dram_tensor("xT", [KB, 128, T], bf16, kind="ExternalInput").ap()
    wg = nc.dram_tensor("wg", [FB, 128, KB * 128], bf16, kind="ExternalInput").ap()
    wu = nc.dram_tensor("wu", [FB, 128, KB * 128], bf16, kind="ExternalInput").ap()
    wd = nc.dram_tensor("wd", [FB, 128, H], bf16, kind="ExternalInput").ap()
    out = nc.dram_tensor("out", [T, H], f32, kind="ExternalOutput").ap()

    with tile.TileContext(nc) as tc, ExitStack() as ctx:
        xpool = ctx.enter_context(tc.tile_pool(name="xpool", bufs=1))
        wpool = ctx.enter_context(tc.tile_pool(name="wpool", bufs=3))
        dpool = ctx.enter_context(tc.tile_pool(name="dpool", bufs=1))
        hpool = ctx.enter_context(tc.tile_pool(name="hpool", bufs=1))
        spool = ctx.enter_context(tc.tile_pool(name="spool", bufs=2))
        opool = ctx.enter_context(tc.tile_pool(name="opool", bufs=3))
        cpool = ctx.enter_context(tc.tile_pool(name="cpool", bufs=1))
        # one psum pool, 4 tags x [128,1024] (2 banks each) = all 8 banks;
        # stage A uses p0/p1 as g/u, stage B uses p0..p3 as 8 accumulators
        psum = ctx.enter_context(tc.tile_pool(name="psum", bufs=1, space="PSUM"))

        bias0 = cpool.tile([128, 1], f32, tag="bias0")
        nc.vector.memset(bias0[:], 0.0)

        # DMA routing: keep the ACT sequencer free of DMA triggers (it must
        # dispatch silu without queueing behind trigger instructions).
        #  - weights (wg/wu + the one-time wd preload) -> sync (SP) HWDGE ring
        #  - activations in (xb) and outputs -> gpsimd (SWDGE), otherwise idle
        # Wd stays resident in SBUF for the whole kernel (2 x 32KB/partition),
        # preloaded during t=0's stage A; stage B never waits on a weight DMA.
        wdp = [
            dpool.tile([128, FB * 512], bf16, tag=f"wdp{h2}", name=f"wdp{h2}")
            for h2 in range(H // 512)
        ]

        for t in range(NT):
            # ---- stage A: hT[f] = silu(Wg_f^T xT) * (Wu_f^T xT), F on partitions
            xb = xpool.tile([128, KB, TB], bf16, tag="xb")
            for k in range(KB):
                if t == 0 and k < 2:
                    # cold start: first two x chunks via the (empty) ACT ring
                    # so the first matmuls aren't gated on SWDGE spin-up
                    nc.scalar.dma_start(xb[:, k, :], xT[k, :, t * TB : (t + 1) * TB])
                else:
                    nc.gpsimd.dma_start(xb[:, k, :], xT[k, :, t * TB : (t + 1) * TB])

            hts = []
            for f in range(FB):
                wgt = wpool.tile([128, KB * 128], bf16, tag="wg")
                if t == 0 and f == 0:
                    # split so the k=0 slice (32KB) lands first
                    for k in range(KB):
                        nc.sync.dma_start(
                            wgt[:, k * 128 : (k + 1) * 128],
                            wg[f][:, k * 128 : (k + 1) * 128],
                        )
                else:
                    nc.sync.dma_start(wgt[:], wg[f])
                wut = wpool.tile([128, KB * 128], bf16, tag="wu")
                nc.sync.dma_start(wut[:], wu[f])
                if t == 0:
                    # interleave the wd preload behind this f's wg/wu so it
                    # never delays stage-A weight prefetch
                    for h2 in range(H // 512):
                        nc.sync.dma_start(
                            wdp[h2][:, f * 512 : (f + 1) * 512],
                            wd[f][:, h2 * 512 : (h2 + 1) * 512],
                        )

                g = psum.tile([128, TB], f32, tag=f"p{(f % 2) * 2}")
                for k in range(KB):
                    for c in range(TB // 512):
                        nc.tensor.matmul(
                            g[:, c * 512 : (c + 1) * 512],
                            wgt[:, k * 128 : (k + 1) * 128],
                            xb[:, k, c * 512 : (c + 1) * 512],
                            start=(k == 0),
                            stop=(k == KB - 1),
                        )
                sil = spool.tile([128, TB], f32, tag="sil")
                nc.scalar.activation(
                    sil[:], g[:], mybir.ActivationFunctionType.Silu, bias=bias0[:]
                )

                u = psum.tile([128, TB], f32, tag=f"p{(f % 2) * 2 + 1}")
                for k in range(KB):
                    for c in range(TB // 512):
                        nc.tensor.matmul(
                            u[:, c * 512 : (c + 1) * 512],
                            wut[:, k * 128 : (k + 1) * 128],
                            xb[:, k, c * 512 : (c + 1) * 512],
                            start=(k == 0),
                            stop=(k == KB - 1),
                        )

                ht = hpool.tile([128, TB], bf16, tag=f"h{f}")
                nc.vector.tensor_mul(ht[:], sil[:], u[:])
                hts.append(ht)

            # ---- stage B: out[tokens, h] += hT^T @ Wd, tokens on partitions
            # single pass over f per h-half: 8 accumulators = 4 psum tiles x 2
            for h2 in range(H // 512):
                accs = [
                    psum.tile([128, TB], f32, tag=f"p{i}", name=f"acc_{h2}_{i}")
                    for i in range(4)
                ]
                for f in range(FB):
                    for m in range(8):
                        nc.tensor.matmul(
                            accs[m // 2][:, (m % 2) * 512 : (m % 2) * 512 + 512],
                            hts[f][:, m * 128 : (m + 1) * 128],
                            wdp[h2][:, f * 512 : (f + 1) * 512],
                            start=(f == 0),
                            stop=(f == FB - 1),
                        )
                for i in range(4):
                    ob = opool.tile([128, TB], f32, tag="ob")
                    nc.vector.tensor_copy(ob[:], accs[i][:])
                    for half in range(2):
                        sl = slice(half * 512, half * 512 + 512)
                        row = t * TB + (2 * i + half) * 128
                        nc.sync.dma_start(
                            out[row : row + 128, h2 * 512 : (h2 + 1) * 512],
                            ob[:, sl],
                        )

    nc.compile()
    return nc


def _get_module():
    if "nc" not in _CACHE:
        _CACHE["nc"] = _build_module()
    return _CACHE["nc"]


def _prep_inputs(hidden_states, Wg, Wu, Wd):
    bf16 = ml_dtypes.bfloat16
    x = np.asarray(hidden_states, dtype=np.float32).reshape(T, H)
    # xT[k, p, t] = x[t, 128k+p]
    xT = np.ascontiguousarray(x.T.reshape(KB, 128, T)).astype(bf16)
    in_maps = []
    for e in range(N_CORES):
        # wg[f, p, (k m)] = Wg[e, 128k+p, 128f+m]
        wg_e = (
            np.asarray(Wg[e], dtype=np.float32)
            .reshape(KB, 128, FB, 128)
            .transpose(2, 1, 0, 3)
            .reshape(FB, 128, KB * 128)
        )
        wu_e = (
            np.asarray(Wu[e], dtype=np.float32)
            .reshape(KB, 128, FB, 128)
            .transpose(2, 1, 0, 3)
            .reshape(FB, 128, KB * 128)
        )
        wd_e = np.asarray(Wd[e], dtype=np.float32).reshape(FB, 128, H)
        in_maps.append(
            {
                "xT": xT,
                "wg": np.ascontiguousarray(wg_e).astype(bf16),
                "wu": np.ascontiguousarray(wu_e).astype(bf16),
                "wd": np.ascontiguousarray(wd_e).astype(bf16),
            }
        )
    return in_maps


def _run(in_maps, trace=False, **kwargs):
    from concourse import bass_utils

    nc = _get_module()
    return bass_utils.run_bass_kernel_spmd(
        nc, in_maps, core_ids=list(range(N_CORES)), trace=trace, **kwargs
    )


def kernel(hidden_states, Wg, Wu, Wd):
    import time

    in_maps = _prep_inputs(hidden_states, Wg, Wu, Wd)
    last_exc = None
    for attempt in range(3):
        try:
            res = _run(in_maps)
            break
        except Exception as exc:  # transient device-unrecoverable wedges
            last_exc = exc
            time.sleep(5 * (attempt + 1))
    else:
        raise last_exc
    partials = np.stack([r["out"] for r in res.results], axis=0)
    total = partials.sum(axis=0, dtype=np.float32)
    return total.reshape(2, 2048, H).astype(np.float32)
